# revision 3
# baseline (speedup 1.0000x reference)
"""ClosestPool1D TRN2 kernel, two-phase exact top-2, software-pipelined.

Phase A (approx): K=15 split-bf16 matmul reconstructs score = 2ab - b2 to
~1e-6 without any ACT/DVE combine (a2 is a per-query constant, ranking-
irrelevant).  A DVE pair-max tree (contiguous groups of G=4 via host-side
DB permutation) reduces 32768 scores/query to an 8192-wide group-max
array; Max8+FindIndex8 per half yields 16 candidate groups (64 candidates).

Phase B (exact): indirect DMA gathers each candidate group's 16-float row
of a [N/4, 16] coord table; a DRAM round-trip re-lays them as a [4, 8192]
rhs; the SAME fp32 K=3 PE matmul the reference jit uses (lhsT=2a.T)
rescores all candidates bit-exactly; ACT assembles -(a2+b2) and a DVE add
reproduces the reference's final subtract bits.  A 2-key (value, index)
arg-select reproduces jax top_k tie semantics exactly.

Pipelining: phase A of m-tile t+1 is emitted before phase B of m-tile t so
the slow strided re-layout DMAs of B overlap the matmul/tree compute of
the next tile (GA is double-buffered to make this legal).
"""
import numpy as np
import ml_dtypes

import concourse.bass as bass
import concourse.mybir as mybir
from concourse.tile import TileContext
from concourse.bass_utils import run_bass_kernel_spmd

f32 = mybir.dt.float32
bf16d = mybir.dt.bfloat16
u32 = mybir.dt.uint32
AFT = mybir.ActivationFunctionType
Alu = mybir.AluOpType
AXX = mybir.AxisListType.X

N = 32768
M = 8192
C = 256
NCORES = 8
MLOC = M // NCORES   # 1024
P = 128
NMT = MLOC // P      # 8 m-tiles
CH = 2048            # approx chunk (psum cols)
NCH = N // CH        # 16
KA = 15              # approx split rows
G = 4                # group size (contiguous orig rows)
NG = N // G          # 8192 groups
NSLOT = 16           # candidate groups per query (8 per half)
CAND = NSLOT * G     # 64 candidates per query
RW = CAND * P        # 8192 rescore columns per m-tile
MMV = 512            # matmul moving size
BIGI = float(1 << 20)


def _split_waits_json(bir_bytes: bytes) -> bytes:
    import orjson

    d = orjson.loads(bir_bytes)
    ctr = [0]

    def mknop(engine, wait, debug):
        ctr[0] += 1
        return {
            "debug": debug,
            "engine": engine,
            "ins": [],
            "name": f"I-waitsplit-{ctr[0]}",
            "opcode": "NoOp",
            "outs": [],
            "sync_info": {"on_update": [], "on_wait": [wait]},
            "text_hint": "waitsplit",
        }

    for f in d.get("functions", []):
        for bb in f.get("blocks", []):
            insts = bb.get("instructions", [])
            out = []
            for i in insts:
                sy = i.get("sync_info")
                if sy:
                    waits = sy.get("on_wait") or []
                    keep = 0 if i.get("opcode") == "ISA" else 1
                    if len(waits) > keep:
                        for w in waits[: len(waits) - keep]:
                            out.append(mknop(i.get("engine"), w, i.get("debug", 0)))
                        sy["on_wait"] = waits[len(waits) - keep:]
                out.append(i)
            bb["instructions"] = out
    return orjson.dumps(d)


def _install_waitsplit():
    import concourse.bass_utils as bu
    import concourse.bass2jax as b2j

    if getattr(bu, "_waitsplit_installed", False):
        return
    orig = bu.compile_bir_kernel

    def patched(bir_json, tmpdir, neff_name="file.neff", **kw):
        return orig(_split_waits_json(bir_json), tmpdir, neff_name, **kw)

    bu.compile_bir_kernel = patched
    b2j.compile_bir_kernel = patched
    bu._waitsplit_installed = True


def _build():
    nc = bass.Bass()
    d = {}
    for s in ("s", "t"):
        d[f"feats_{s}"] = nc.dram_tensor(f"feats_{s}", [N, C], f32, kind="ExternalInput")
        d[f"btab_{s}"] = nc.dram_tensor(f"btab_{s}", [NG, 16], f32, kind="ExternalInput")
        d[f"V_{s}"] = nc.dram_tensor(f"V_{s}", [KA, N], bf16d, kind="ExternalInput")
        d[f"U_{s}"] = nc.dram_tensor(f"U_{s}", [KA, MLOC], bf16d, kind="ExternalInput")
        for k in range(G):
            d[f"x2p{k}_{s}"] = nc.dram_tensor(
                f"x2p{k}_{s}", [12, MLOC], f32, kind="ExternalInput")
        d[f"na2_{s}"] = nc.dram_tensor(f"na2_{s}", [P, NMT], f32, kind="ExternalInput")
        d[f"out_{s}"] = nc.dram_tensor(f"out_{s}", [MLOC, C], f32, kind="ExternalOutput")
        for t in range(NMT):
            d[f"rgd_{s}{t}"] = nc.dram_tensor(
                f"rgd_{s}{t}", [P, NSLOT, 16], f32, kind="Internal")
            d[f"mmd_{s}{t}"] = nc.dram_tensor(
                f"mmd_{s}{t}", [P * (RW + CAND)], f32, kind="Internal")

    with TileContext(nc) as tc:
        with (
            tc.tile_pool(name="const", bufs=1) as cp,
            tc.tile_pool(name="vin", bufs=6) as vp,
            tc.tile_pool(name="ga", bufs=2) as gap,
            tc.tile_pool(name="tree", bufs=3) as trp,
            tc.tile_pool(name="small", bufs=4) as sp,
            tc.tile_pool(name="resc", bufs=2) as rp,
            tc.tile_pool(name="ps", bufs=2, space="PSUM") as psp,
        ):
            utiles = {}
            x2tiles = {}
            na2tiles = {}
            for s in ("s", "t"):
                ut = cp.tile([KA, MLOC], bf16d, tag=f"U_{s}")
                nc.sync.dma_start(ut[:], d[f"U_{s}"][:])
                utiles[s] = ut
                xts = []
                for k in range(G):
                    xt = cp.tile([12, MLOC], f32, tag=f"x2p{k}_{s}")
                    nc.sync.dma_start(xt[:], d[f"x2p{k}_{s}"][:])
                    xts.append(xt)
                x2tiles[s] = xts
                nt = cp.tile([P, NMT], f32, tag=f"na2_{s}")
                nc.sync.dma_start(nt[:], d[f"na2_{s}"][:])
                na2tiles[s] = nt

            def phase_a(s, t):
                """Approx scores + group-max tree + slot selection."""
                lhsA = utiles[s][:, t * P:(t + 1) * P]
                ga = gap.tile([P, NG], f32, tag="GA")
                for q in range(NCH):
                    vq = vp.tile([KA, CH], bf16d, tag="Vq")
                    nc.sync.dma_start(vq[:], d[f"V_{s}"][:, q * CH:(q + 1) * CH])
                    pa = psp.tile([P, CH], f32, tag="ps")
                    for c in range(CH // MMV):
                        nc.tensor.matmul(
                            pa[:, c * MMV:(c + 1) * MMV],
                            lhsT=lhsA,
                            rhs=vq[:, c * MMV:(c + 1) * MMV],
                            start=True, stop=True)
                    up = trp.tile([P, CH // 2], f32, tag="UP")
                    nc.scalar.copy(up[:], pa[:, CH // 2:CH])
                    t1 = trp.tile([P, CH // 2], f32, tag="T1")
                    nc.vector.tensor_tensor(
                        t1[:], up[:], pa[:, 0:CH // 2], op=Alu.max)
                    nc.vector.tensor_tensor(
                        ga[:, q * (CH // 4):(q + 1) * (CH // 4)],
                        t1[:, 0:CH // 4], t1[:, CH // 4:CH // 2], op=Alu.max)
                # top-8 groups per half -> 16 slots
                sg = sp.tile([P, NSLOT], f32, tag="sg")
                for h in range(2):
                    hv = sp.tile([P, 8], f32, tag="hv")
                    nc.vector.max(
                        out=hv[:], in_=ga[:, h * (NG // 2):(h + 1) * (NG // 2)])
                    hi = sp.tile([P, 8], u32, tag="hi")
                    nc.vector.max_index(
                        out=hi[:], in_max=hv[:],
                        in_values=ga[:, h * (NG // 2):(h + 1) * (NG // 2)])
                    hif = sp.tile([P, 8], f32, tag="hif")
                    nc.vector.tensor_copy(hif[:], hi[:])
                    nc.vector.tensor_scalar(
                        sg[:, h * 8:(h + 1) * 8], hif[:],
                        float(h * (NG // 2)), None, op0=Alu.add)
                # gather candidate groups + start the DRAM round-trip early
                sgu = sp.tile([P, NSLOT], u32, tag="sgu")
                nc.vector.tensor_copy(sgu[:], sg[:])
                rg = rp.tile([P, NSLOT * 16], f32, tag="rg")
                for sl in range(NSLOT):
                    nc.gpsimd.indirect_dma_start(
                        out=rg[:, sl * 16:(sl + 1) * 16],
                        out_offset=None,
                        in_=d[f"btab_{s}"][:],
                        in_offset=bass.IndirectOffsetOnAxis(
                            ap=sgu[:, sl:sl + 1], axis=0),
                    )
                rgd = d[f"rgd_{s}{t}"]
                nc.scalar.dma_start(rgd[:, :, :], rg[:])
                # rhs12 [12, 2048]: col (m, slot) = the group's 12 coord
                # floats (dd-major, 48B contiguous in DRAM -> fills 12
                # partitions per fragment); split by m-half across queues
                rhsT = rp.tile([12, P * NSLOT], f32, tag="rhsT")
                engs = (nc.sync, nc.scalar)
                for mh in range(4):
                    engs[mh % 2].dma_start(
                        rhsT[:, mh * (P * NSLOT // 4):(mh + 1) * (P * NSLOT // 4)],
                        rgd[mh * (P // 4):(mh + 1) * (P // 4), :, 0:12]
                        .transpose([2, 0, 1]).opt())
                # b2 per candidate from SBUF (no DMA): c = k*16 + slot
                b2c = rp.tile([P, CAND], f32, tag="b2c")
                rg3 = rg[:].rearrange("p (s e) -> p s e", s=NSLOT, e=16)
                nc.vector.tensor_copy(
                    b2c[:], rg3[:, :, 12:16].transpose([0, 2, 1]).opt())
                # candidate orig index table: idx = 4*gid + k
                idxf = rp.tile([P, CAND], f32, tag="idxf")
                sg4 = sp.tile([P, NSLOT], f32, tag="sg4")
                nc.vector.tensor_scalar(sg4[:], sg[:], 4.0, None, op0=Alu.mult)
                for k in range(G):
                    nc.vector.tensor_scalar(
                        idxf[:, k * NSLOT:(k + 1) * NSLOT], sg4[:],
                        float(k), None, op0=Alu.add)
                return ga, rhsT, b2c, idxf

            def phase_b(s, t, state):
                """Exact fp32 PE rescore of the 64 candidates + selection."""
                ga, rhsT, b2c, idxf = state
                mmf = d[f"mmd_{s}{t}"]
                wv = mmf[0:P * RW].rearrange("(p c) -> p c", p=P)
                for blk in range(RW // CH):
                    lhsX = x2tiles[s][blk][:, t * P:(t + 1) * P]
                    pb = psp.tile([P, CH], f32, tag="ps")
                    for c in range(CH // MMV):
                        off = c * MMV
                        nc.tensor.matmul(
                            pb[:, c * MMV:(c + 1) * MMV],
                            lhsT=lhsX,
                            rhs=rhsT[0:12, off:off + MMV],
                            start=True, stop=True)
                    mm_sb = trp.tile([P, CH], f32, tag="mmsb")
                    nc.scalar.copy(mm_sb[:], pb[:])
                    nc.scalar.dma_start(wv[:, blk * CH:(blk + 1) * CH], mm_sb[:])
                # diagonal re-read: mm(m, k, s) = flat[m*8208 + k*2048 + s]
                mmc = rp.tile([P, CAND], f32, tag="mmc")
                rv = (mmf[0:P * (RW + NSLOT)]
                      .rearrange("(p x) -> p x", p=P)[:, 0:RW]
                      .rearrange("p (k s8) -> p k s8", k=G, s8=P * NSLOT)
                      [:, :, 0:NSLOT])
                nc.sync.dma_start(
                    mmc[:].rearrange("p (k s) -> p k s", k=G, s=NSLOT).opt(),
                    rv.opt())
                # t1n = -(a2 + b2) with reference rounding; combine
                t1n = rp.tile([P, CAND], f32, tag="t1n")
                nc.scalar.activation(
                    t1n[:], b2c[:], AFT.Identity,
                    bias=na2tiles[s][:, t:t + 1], scale=-1.0)
                sc = rp.tile([P, CAND], f32, tag="sc")
                nc.vector.tensor_tensor(sc[:], t1n[:], mmc[:], op=Alu.add)
                # 2-key (value desc, index asc) top-2
                m1 = sp.tile([P, 8], f32, tag="m1")
                nc.vector.max(out=m1[:], in_=sc[:])
                eq1 = rp.tile([P, CAND], f32, tag="eq1")
                nc.vector.tensor_tensor(
                    eq1[:], sc[:], m1[:, 0:1].to_broadcast([P, CAND]),
                    op=Alu.is_equal)
                idshift = rp.tile([P, CAND], f32, tag="idshift")
                nc.vector.tensor_scalar(
                    idshift[:], idxf[:], -BIGI, None, op0=Alu.add)
                t3 = rp.tile([P, CAND], f32, tag="t3")
                nc.vector.tensor_tensor(t3[:], idshift[:], eq1[:], op=Alu.mult)
                p1 = sp.tile([P, 1], f32, tag="p1")
                nc.vector.tensor_reduce(p1[:], t3[:], AXX, Alu.min)
                excl = rp.tile([P, CAND], f32, tag="excl")
                nc.vector.tensor_tensor(
                    excl[:], idshift[:], p1[:, 0:1].to_broadcast([P, CAND]),
                    op=Alu.is_equal)
                pen = rp.tile([P, CAND], f32, tag="pen")
                nc.vector.tensor_scalar(pen[:], excl[:], 1e30, None, op0=Alu.mult)
                sc2 = rp.tile([P, CAND], f32, tag="sc2")
                nc.vector.tensor_tensor(sc2[:], sc[:], pen[:], op=Alu.subtract)
                m2 = sp.tile([P, 8], f32, tag="m2")
                nc.vector.max(out=m2[:], in_=sc2[:])
                eq2 = rp.tile([P, CAND], f32, tag="eq2")
                nc.vector.tensor_tensor(
                    eq2[:], sc2[:], m2[:, 0:1].to_broadcast([P, CAND]),
                    op=Alu.is_equal)
                t4 = rp.tile([P, CAND], f32, tag="t4")
                nc.vector.tensor_tensor(t4[:], idshift[:], eq2[:], op=Alu.mult)
                p2 = sp.tile([P, 1], f32, tag="p2")
                nc.vector.tensor_reduce(p2[:], t4[:], AXX, Alu.min)
                p2i = sp.tile([P, 1], f32, tag="p2i")
                nc.vector.tensor_scalar(p2i[:], p2[:], BIGI, None, op0=Alu.add)
                p2u = sp.tile([P, 1], u32, tag="p2u")
                nc.vector.tensor_copy(p2u[:], p2i[:])
                g = sp.tile([P, C], f32, tag="g")
                nc.gpsimd.indirect_dma_start(
                    out=g[:],
                    out_offset=None,
                    in_=d[f"feats_{s}"][:],
                    in_offset=bass.IndirectOffsetOnAxis(ap=p2u[:, :1], axis=0),
                )
                nc.scalar.dma_start(d[f"out_{s}"][t * P:(t + 1) * P, :], g[:])

            tiles = [(s, t) for s in ("s", "t") for t in range(NMT)]
            pending = None
            for s, t in tiles:
                state = phase_a(s, t)
                if pending is not None:
                    phase_b(*pending)
                pending = (s, t, state)
            phase_b(*pending)
    return nc


_NC_CACHE = {}


def _get_nc():
    if "nc" not in _NC_CACHE:
        _install_waitsplit()
        _NC_CACHE["nc"] = _build()
    return _NC_CACHE["nc"]


def _prep_side(feats, bcoord, acoord):
    bf = ml_dtypes.bfloat16
    b = bcoord.astype(np.float32)
    b2 = (b[:, 0] * b[:, 0] + b[:, 1] * b[:, 1]).astype(np.float32)
    b2 = (b2 + b[:, 2] * b[:, 2]).astype(np.float32)
    # kernel col p = (q, i, h) -> orig row 4*(512q + i) + h
    n = np.arange(N)
    gid = n // 4
    q = gid // (CH // 4)
    i = gid % (CH // 4)
    h = n % 4
    pos = q * CH + i + (CH // 4) * h
    perm = np.empty(N, np.int64)
    perm[pos] = n
    bp = b[perm]
    b2p = b2[perm]
    b_hi = bp.astype(bf).astype(np.float32)
    b_lo = (bp - b_hi).astype(bf)
    b2h = b2p.astype(bf).astype(np.float32)
    b2l = (b2p - b2h).astype(bf)
    b2l2 = (b2p - b2h - b2l.astype(np.float32)).astype(bf)
    V = np.zeros((KA, N), bf)
    for dd in range(3):
        V[dd] = b_hi[:, dd].astype(bf)
        V[3 + dd] = b_hi[:, dd].astype(bf)
        V[6 + dd] = b_lo[:, dd]
        V[9 + dd] = b_lo[:, dd]
    V[12] = (-b2h).astype(bf)
    V[13] = -b2l
    V[14] = -b2l2
    btab = np.concatenate(
        [bp0.reshape(NG, 4) for bp0 in (b[:, 0], b[:, 1], b[:, 2], b2)], axis=1)
    shared = {
        "feats": np.ascontiguousarray(feats),
        "btab": np.ascontiguousarray(btab.astype(np.float32)),
        "V": np.ascontiguousarray(V),
    }
    a = acoord.astype(np.float32)
    x2 = (2.0 * a).astype(np.float32)
    x_hi = x2.astype(bf).astype(np.float32)
    x_lo = (x2 - x_hi).astype(bf)
    a2 = (a[:, 0] * a[:, 0] + a[:, 1] * a[:, 1]).astype(np.float32)
    a2 = (a2 + a[:, 2] * a[:, 2]).astype(np.float32)
    per_core = []
    for cix in range(NCORES):
        sl = slice(cix * MLOC, (cix + 1) * MLOC)
        U = np.zeros((KA, MLOC), bf)
        for dd in range(3):
            U[dd] = x_hi[sl, dd].astype(bf)
            U[3 + dd] = x_lo[sl, dd]
            U[6 + dd] = x_hi[sl, dd].astype(bf)
            U[9 + dd] = x_lo[sl, dd]
        U[12:15] = np.ones((3, MLOC), bf)
        na2 = (-a2[sl]).reshape(NMT, P).T
        pc = {
            "U": np.ascontiguousarray(U),
            "na2": np.ascontiguousarray(na2.astype(np.float32)),
        }
        for k in range(G):
            xp = np.zeros((12, MLOC), np.float32)
            for dd in range(3):
                xp[dd * 4 + k] = x2[sl, dd]
            pc[f"x2p{k}"] = xp
        per_core.append(pc)
    return shared, per_core


def kernel(src, tgt, src_coords, tgt_coords, src_shortcut_coords, tgt_shortcut_coords):
    src = np.ascontiguousarray(np.asarray(src, np.float32))
    tgt = np.ascontiguousarray(np.asarray(tgt, np.float32))
    nc = _get_nc()

    sh_s, pc_s = _prep_side(src, np.asarray(src_coords, np.float32),
                            np.asarray(src_shortcut_coords, np.float32))
    sh_t, pc_t = _prep_side(tgt, np.asarray(tgt_coords, np.float32),
                            np.asarray(tgt_shortcut_coords, np.float32))

    in_maps = []
    for cix in range(NCORES):
        m = {}
        for s, sh, pc in (("s", sh_s, pc_s), ("t", sh_t, pc_t)):
            m[f"feats_{s}"] = sh["feats"]
            m[f"btab_{s}"] = sh["btab"]
            m[f"V_{s}"] = sh["V"]
            m[f"U_{s}"] = pc[cix]["U"]
            for k in range(G):
                m[f"x2p{k}_{s}"] = pc[cix][f"x2p{k}"]
            m[f"na2_{s}"] = pc[cix]["na2"]
        in_maps.append(m)

    import os
    import time as _time
    trace = bool(os.environ.get("KERNEL_TRACE"))
    last_err = None
    for _attempt in range(3):
        try:
            r = run_bass_kernel_spmd(
                nc, in_maps, core_ids=list(range(NCORES)), trace=trace)
            break
        except Exception as e:
            last_err = e
            _time.sleep(3.0)
    else:
        raise last_err
    LAST_RESULTS["r"] = r
    res = r.results
    out_src = np.concatenate([res[cix]["out_s"] for cix in range(NCORES)], axis=0)
    out_tgt = np.concatenate([res[cix]["out_t"] for cix in range(NCORES)], axis=0)
    return (out_src, out_tgt)


LAST_RESULTS = {}
